# revision 12
# baseline (speedup 1.0000x reference)
"""GroupedMoE Trainium2 kernel.

Strategy (expert-parallel, per sharding hint):
  - Host: router (sigmoid + top-6), gating softmax (bf16), capacity-768
    dispatch into a padded per-expert buffer, final combine/scatter-add.
  - Device (8 cores): each core runs 8 expert slots (experts assigned by
    snake/balanced order over per-expert token counts) computing the
    SwiGLU grouped GEMM, plus 1/8 of the tokens for the dense shared
    expert (4 d_ff slices accumulated on-chip).
  - All GEMMs run in bf16 with fp32 PSUM accumulation.  Weights stream
    from HBM in natural layout; the dispatch buffer is fed transposed
    ([D, tokens]) so no on-chip transposes are needed anywhere.
"""

import numpy as np
import ml_dtypes

import concourse.bass as bass
import concourse.mybir as mybir
from concourse.tile import TileContext
from concourse.bass_utils import run_bass_kernel_spmd

B, S, D = 2, 2048, 1024
E, K, F = 64, 6, 1408
DFF = 4096
N = B * S
CAP = 768
NCORES = 8
NSLOT = E // NCORES  # 8 expert slots per core
TOK_C = N // NCORES  # 512 shared-expert tokens per core
NKD = D // 128       # 8 contraction chunks over D

BF16 = ml_dtypes.bfloat16
LAST_RESULTS = None  # test harness introspection

# walrus codegen fits only N sync waits in these instruction encodings;
# split any excess into standalone EventSemaphore waits on the same engine
# (executed in-order by the sequencer right before the instruction).
_WAIT_LIMITS = (
    (mybir.InstActivation, 1),
    (mybir.InstTensorTensor, 1),
    (mybir.InstTensorCopy, 1),
    (mybir.InstMatmult, 1),
    (mybir.InstLdweights, 1),
    (mybir.InstMemset, 1),
    (mybir.InstDMACopy, 1),
    (mybir.InstDrain, 1),
)
_ORIG_COMMIT = TileContext._commit_instruction


def _split_commit(self, inst, lazy_reg_writes=True):
    si = inst.sync_info
    if si is not None and si.on_wait:
        lim = next((l for t, l in _WAIT_LIMITS if isinstance(inst, t)), None)
        if lim is not None and len(si.on_wait) > lim:
            waits = list(si.on_wait)
            for w in waits[:-lim]:
                ev = mybir.InstEventSemaphore(
                    name=self.nc.get_next_instruction_name(),
                    engine=inst.engine,
                )
                ev.sync_info = mybir.SyncInfo(on_wait=[w], on_update=[])
                _ORIG_COMMIT(self, ev, lazy_reg_writes=False)
            inst.sync_info = mybir.SyncInfo(
                on_wait=waits[-lim:], on_update=list(si.on_update)
            )
    return _ORIG_COMMIT(self, inst, lazy_reg_writes)


TileContext._commit_instruction = _split_commit

_ORIG_DNB = TileContext._drain_and_barrier


def _patched_dnb(self, tick_clock, wait_clock):
    # the kernel-tail drain accumulates one wait per active proc (>10);
    # hoist them onto standalone SP event-sem waits executed just before it
    from concourse.vector_clock import ScopedClock

    probe = mybir.InstNoOp(
        name=self.nc.get_next_instruction_name(), engine=mybir.EngineType.SP
    )
    wait_clock.add_sem_waits(probe, ScopedClock({None: tick_clock.global_clock}))
    waits = list(probe.sync_info.on_wait) if probe.sync_info else []
    for w in waits:
        ev = mybir.InstEventSemaphore(
            name=self.nc.get_next_instruction_name(), engine=mybir.EngineType.SP
        )
        ev.sync_info = mybir.SyncInfo(on_wait=[w], on_update=[])
        self._add_instruction(ev)
    self.nc.sync.drain()
    self.nc.all_engine_barrier()
    popped = self.nc._tile_sem_poison_stack.pop()
    assert popped is self._sem_poison
    self.nc.clear_and_free_semaphores(list(self.sems.allocated().values()))
    self.nc.all_engine_barrier()


TileContext._drain_and_barrier = _patched_dnb


def _route(h, router_w):
    """Replicates the reference router + dispatch bookkeeping in numpy."""
    logits = h @ router_w.T                      # [N, E] fp32
    scores = 1.0 / (1.0 + np.exp(-logits))
    idx_part = np.argpartition(-scores, K - 1, axis=1)[:, :K]
    part_scores = np.take_along_axis(scores, idx_part, axis=1)
    ordr = np.lexsort((idx_part, -part_scores), axis=1)
    topk_idx = np.take_along_axis(idx_part, ordr, axis=1)        # [N, K]
    topk_logits = np.take_along_axis(logits, topk_idx, axis=1)
    mx = topk_logits.max(1, keepdims=True)
    ex = np.exp(topk_logits - mx)
    gating = (ex / ex.sum(1, keepdims=True)).astype(BF16)        # [N, K] bf16

    flat_idx = topk_idx.reshape(-1)
    order = np.argsort(flat_idx, kind="stable")
    counts = np.bincount(flat_idx, minlength=E)
    starts = np.concatenate([[0], np.cumsum(counts)[:-1]])
    token_ids = np.repeat(np.arange(N), K)[order]
    inv = np.empty_like(order)
    inv[order] = np.arange(N * K)
    return logits, gating, order, inv, counts, starts, token_ids


def _build_program(chunks):
    """One SPMD program for all 8 cores; chunks[s] = 128-row tiles in slot s."""
    nc = bass.Bass("TRN2")
    bf = mybir.dt.bfloat16
    f32 = mybir.dt.float32
    AF = mybir.ActivationFunctionType

    w12_d = nc.dram_tensor("w12", [NSLOT, 2, D, F], bf, kind="ExternalInput")
    w3_d = nc.dram_tensor("w3", [NSLOT, F, D], bf, kind="ExternalInput")
    xt_d = [
        nc.dram_tensor(f"xt{s}", [D, chunks[s] * 128], bf, kind="ExternalInput")
        for s in range(NSLOT)
    ]
    guw_d = nc.dram_tensor("guw", [2, 4, D, DFF // 4], bf, kind="ExternalInput")
    dwt_d = nc.dram_tensor("dwt", [4, DFF // 4, D], bf, kind="ExternalInput")
    xst_d = nc.dram_tensor("xst", [D, TOK_C], bf, kind="ExternalInput")
    eout_d = [
        nc.dram_tensor(f"eout{s}", [chunks[s] * 128, D], f32, kind="ExternalOutput")
        for s in range(NSLOT)
    ]
    sout_d = nc.dram_tensor("sout", [TOK_C, D], f32, kind="ExternalOutput")

    with TileContext(nc) as tc:
        with (
            tc.tile_pool(name="wp", bufs=4) as wp,        # weight tiles, 22.5KB/part
            tc.tile_pool(name="xtp", bufs=2) as xtp,      # dispatch tiles
            tc.tile_pool(name="xsp", bufs=1) as xsp,      # shared-expert tokens
            tc.tile_pool(name="actp", bufs=2) as actp,    # swiglu activations
            tc.tile_pool(name="tmpp", bufs=2) as tmpp,    # silu f32 staging
            tc.tile_pool(name="outp", bufs=2) as outp,    # psum eviction staging
            tc.tile_pool(name="sap", bufs=4) as sap,      # shared-expert accumulators
            tc.tile_pool(name="psp", bufs=8, space="PSUM") as psp,
        ):
            def ffn_slot(w1_ap, w2_ap, w3_ap, fh, xt_sb, ts_rows, out_dram, sacc):
                """SwiGLU FFN for one slot: out = (silu(x@w1)*(x@w2)) @ w3.

                w1_ap/w2_ap: DRAM [D, fh]; w3_ap: DRAM [fh, D]; xt_sb: SBUF
                [128, NKD, ts_rows].  Writes to out_dram rows (experts) or
                accumulates into sacc tiles (shared expert slices).
                """
                nf = fh // 128
                w1_sb = wp.tile([128, NKD, fh], bf, tag="w")
                w2_sb = wp.tile([128, NKD, fh], bf, tag="w")
                w3_sb = wp.tile([128, nf, D], bf, tag="w")
                h1 = (fh // 256) * 128  # split weight DMAs so j=0 work starts early
                for (dst, src) in ((w1_sb, w1_ap), (w2_sb, w2_ap)):
                    ap = src.rearrange("(k p) f -> p k f", p=128)
                    nc.sync.dma_start(dst[:, :, 0:h1], ap[:, :, 0:h1])
                    nc.sync.dma_start(dst[:, :, h1:fh], ap[:, :, h1:fh])
                nc.sync.dma_start(
                    w3_sb[:], w3_ap.rearrange("(k p) f -> p k f", p=128)
                )
                for off in range(0, ts_rows, 512):
                    nt = min(512, ts_rows - off)
                    act_sb = actp.tile([128, nf, nt], bf, tag="act")
                    for j in range(nf):
                        pa = psp.tile([128, nt], f32, tag="ps")
                        pb = psp.tile([128, nt], f32, tag="ps")
                        for k in range(NKD):
                            nc.tensor.matmul(
                                pa[:],
                                w1_sb[:, k, j * 128:(j + 1) * 128],
                                xt_sb[:, k, off:off + nt],
                                start=(k == 0), stop=(k == NKD - 1),
                            )
                        for k in range(NKD):
                            nc.tensor.matmul(
                                pb[:],
                                w2_sb[:, k, j * 128:(j + 1) * 128],
                                xt_sb[:, k, off:off + nt],
                                start=(k == 0), stop=(k == NKD - 1),
                            )
                        tmp = tmpp.tile([128, nt], f32, tag="tmp")
                        pbs = tmpp.tile([128, nt], f32, tag="tmp2")
                        # in-place self-copy fences: each engine-side op below
                        # must fit its ISA sync-wait slot limit (AC: 2, TT: 1),
                        # so fences absorb slot-release / cross-engine waits
                        nc.scalar.activation(tmp[:, 0:1], tmp[:, 0:1], AF.Copy)
                        nc.scalar.activation(tmp[:], pa[:], AF.Silu)
                        nc.vector.tensor_copy(pbs[:], pb[:])
                        nc.vector.tensor_copy(tmp[:, 0:1], tmp[:, 0:1])
                        nc.vector.tensor_mul(act_sb[:, j, :], tmp[:], pbs[:])
                    for t in range(nt // 128):
                        if out_dram is not None:
                            out_sb = outp.tile([128, D], f32, tag="out")
                            nc.scalar.activation(
                                out_sb[:, 0:1], out_sb[:, 0:1], AF.Copy
                            )
                        for n in range(D // 512):
                            po = psp.tile([128, 512], f32, tag="ps")
                            for k in range(nf):
                                nc.tensor.matmul(
                                    po[:],
                                    act_sb[:, k, t * 128:(t + 1) * 128],
                                    w3_sb[:, k, n * 512:(n + 1) * 512],
                                    start=(k == 0), stop=(k == nf - 1),
                                )
                            if out_dram is not None:
                                nc.scalar.activation(
                                    out_sb[:, n * 512:(n + 1) * 512], po[:], AF.Copy
                                )
                            else:
                                acc = sacc[(off + t * 128) // 128]
                                nc.vector.tensor_add(
                                    acc[:, n * 512:(n + 1) * 512],
                                    acc[:, n * 512:(n + 1) * 512],
                                    po[:],
                                )
                        if out_dram is not None:
                            nc.sync.dma_start(
                                out_dram[off + t * 128: off + (t + 1) * 128, :],
                                out_sb[:],
                            )

            for s in range(NSLOT):
                ts_rows = chunks[s] * 128
                xt_sb = xtp.tile([128, NKD, ts_rows], bf, tag="xt")
                nc.sync.dma_start(
                    xt_sb[:], xt_d[s].rearrange("(k p) t -> p k t", p=128)
                )
                ffn_slot(
                    w12_d[s, 0], w12_d[s, 1], w3_d[s], F,
                    xt_sb, ts_rows, eout_d[s], None,
                )

            # shared expert: 4 d_ff slices accumulated into sacc, then stored
            xst_sb = xsp.tile([128, NKD, TOK_C], bf, tag="xs")
            nc.sync.dma_start(
                xst_sb[:], xst_d.rearrange("(k p) t -> p k t", p=128)
            )
            sacc = []
            for t in range(TOK_C // 128):
                acc_t = sap.tile([128, D], f32, tag="sa")
                sacc.append(acc_t)
            for acc in sacc:
                nc.vector.memset(acc[:], 0.0)
            for p in range(4):
                ffn_slot(
                    guw_d[0, p], guw_d[1, p], dwt_d[p], DFF // 4,
                    xst_sb, TOK_C, None, sacc,
                )
            for t in range(TOK_C // 128):
                nc.sync.dma_start(sout_d[t * 128:(t + 1) * 128, :], sacc[t][:])

    return nc


def _ensure_ntff_hook():
    """Some containers ship concourse without antenv.axon_hooks; provide it
    so run_bass_kernel_spmd(trace=True) can profile instead of crashing."""
    try:
        import antenv.axon_hooks  # noqa: F401
        return
    except ImportError:
        pass
    import sys
    import types

    try:
        import antenv
    except ImportError:
        antenv = types.ModuleType("antenv")
        sys.modules["antenv"] = antenv
    mod = types.ModuleType("antenv.axon_hooks")
    holder = {"hook": None}
    mod.set_axon_ntff_profile_hook = lambda h: holder.__setitem__("hook", h)
    mod.get_axon_ntff_profile_hook = lambda: holder["hook"]
    sys.modules["antenv.axon_hooks"] = mod
    antenv.axon_hooks = mod
    try:
        from trn_agent_boot.trn_boot import _ntff_profile_via_ctypes

        hook = _ntff_profile_via_ctypes("/opt/axon/libaxon_pjrt.so")
        if hook is not None:
            mod.set_axon_ntff_profile_hook(hook)
    except Exception:
        pass


def kernel(x, router_w, experts_w12, experts_w3, gate_w, up_w, down_w):
    global LAST_RESULTS
    _ensure_ntff_hook()
    x = np.asarray(x, np.float32)
    h = x.reshape(-1, D)
    logits, gating, order, inv, counts, starts, token_ids = _route(
        h, np.asarray(router_w, np.float32)
    )

    cnt_cap = np.minimum(counts, CAP)
    # balanced slot assignment: sort experts by load, slot s gets the 8
    # experts ranked [8s, 8s+8) so every core's slot-s work is similar
    exp_order = np.argsort(-cnt_cap, kind="stable")
    perm = exp_order.reshape(NSLOT, NCORES)            # perm[s][c] = expert id
    chunks = [
        max(1, int(np.ceil(cnt_cap[perm[s]].max() / 128))) for s in range(NSLOT)
    ]

    hb = h.astype(BF16)
    w12b = np.asarray(experts_w12, np.float32).astype(BF16)   # [E, D, 2F]
    w3b = np.asarray(experts_w3, np.float32).astype(BF16)     # [E, F, D]
    gT = np.ascontiguousarray(np.asarray(gate_w, np.float32).T.astype(BF16))
    uT = np.ascontiguousarray(np.asarray(up_w, np.float32).T.astype(BF16))
    dT = np.ascontiguousarray(np.asarray(down_w, np.float32).T.astype(BF16))
    guw = np.stack([
        gT.reshape(D, 4, DFF // 4).transpose(1, 0, 2),
        uT.reshape(D, 4, DFF // 4).transpose(1, 0, 2),
    ]).copy()                                                 # [2, 4, D, DFF/4]
    dwt = dT.reshape(4, DFF // 4, D).copy()                   # [4, DFF/4, D]

    in_maps = []
    for c in range(NCORES):
        m = {}
        w12c = np.empty((NSLOT, 2, D, F), BF16)
        w3c = np.empty((NSLOT, F, D), BF16)
        for s in range(NSLOT):
            e = perm[s][c]
            w12c[s, 0] = w12b[e][:, :F]
            w12c[s, 1] = w12b[e][:, F:]
            w3c[s] = w3b[e]
            ts_rows = chunks[s] * 128
            xt = np.zeros((D, ts_rows), BF16)
            cnt = int(cnt_cap[e])
            rows = token_ids[starts[e]: starts[e] + cnt]
            xt[:, :cnt] = hb[rows].T
            m[f"xt{s}"] = xt
        m["w12"] = w12c
        m["w3"] = w3c
        m["guw"] = guw
        m["dwt"] = dwt
        m["xst"] = np.ascontiguousarray(hb[c * TOK_C:(c + 1) * TOK_C].T)
        in_maps.append(m)

    nc = _build_program(chunks)
    res = run_bass_kernel_spmd(nc, in_maps, list(range(NCORES)))
    LAST_RESULTS = res

    # combine: weighted scatter-add of expert rows back to token order
    full = np.zeros((N * K, D), np.float32)
    for s in range(NSLOT):
        for c in range(NCORES):
            e = perm[s][c]
            cnt = int(cnt_cap[e])
            full[starts[e]: starts[e] + cnt] = res.results[c][f"eout{s}"][:cnt]
    g_sorted = gating.reshape(-1)[order].astype(np.float32)
    weighted = full * g_sorted[:, None]
    routed = weighted[inv].reshape(N, K, D).sum(1)
    shared = np.concatenate([res.results[c]["sout"] for c in range(NCORES)], 0)
    out = ((shared + routed) * 0.5).reshape(B, S, D).astype(np.float32)
    z_loss = np.float32(np.mean(logits.astype(np.float32) ** 2) * 1e-6)
    return out, z_loss
